# revision 21
# baseline (speedup 1.0000x reference)
"""Trainium2 Bass kernel for BilateralSlicer (fused trilinear bilateral-grid slice).

out[b,c,y,x]: spatial bilinear upsample of a 16x16x8-deep grid to 1080x1920
with per-pixel depth lerp driven by guidance.

v2 strategy (8 cores, full I/O):
  - Shard the 2*1080 = 2160 output rows: core i gets rows [i*270, (i+1)*270).
  - Host precomputes, per output row, a 128-entry table indexed by
    e = 16*klo + jlo, each entry a 256B chunk of 64 fp32:
      [T(c), DjT(c), TD(c), DjTD(c)] x 12 channels (+16 pad), where
      T = y-lerped grid, TD = depth diff, Dj* = x-neighbor diffs.
  - Device, per 8-row block (pixel stream i = g*1920 + x, 15360 pixels):
      E = gpsimd.dma_gather(T4_block, idx)   # SDMA gather, pixel i -> partition i%128
      f = frac(7*guidance)                   # DVE, per-pixel (partition,slot)
      O = (T + wx*DjT) + f*(TD + wx*DjTD)    # 6 DVE ops w/ broadcast views
      PE-transpose O (per channel) -> (slot, c, lane) and DMA out contiguously.
  - Host computes gather indices (idx = 128*g + 16*klo + jlo, klo via the
    same fp32 round-to-even magic the device uses for f) and unshards.

Self-contained: hardcodes all shapes; needs numpy + the in-container
concourse (bass) stack.
"""

import os
import sys
import numpy as np

for _p in ("/opt/trn_rl_repo",):
    if _p not in sys.path:
        sys.path.insert(0, _p)

import concourse.bass as bass
import concourse.mybir as mybir
from concourse import bacc
from concourse import tile
from concourse import library_config
from concourse.bass_utils import run_bass_kernel_spmd

F32 = mybir.dt.float32
I16 = mybir.dt.int16

B, C, D, HG, WG = 2, 12, 8, 16, 16
HH, WH = 1080, 1920
NCORES = 8
ROWS_TOTAL = B * HH            # 2160
RPC = ROWS_TOTAL // NCORES     # 270 rows per core
BLK = 8                        # rows per device block
NBLK = (RPC + BLK - 1) // BLK  # 34 (last block has 6 active rows)
NE = D * WG                    # 128 table entries (k,j) per row
ELEM = 64                      # fp32 per chunk (256 B)
NPIX = BLK * WH                # 15360 pixels per full block
NB = NPIX // 128               # 120 slots per partition per full block
XB = WH // 128                 # 15 lane-groups per row
MAGIC = np.float32(12582912.0)  # 1.5 * 2**23

LAST_EXEC_NS = None
LAST_PROFILE = None


def _src_coords(out_size, in_size):
    """fp32 mirror of reference._src_coords (PyTorch bilinear, align_corners=False)."""
    scale = np.float32(in_size / out_size)
    src = (np.arange(out_size, dtype=np.float32) + np.float32(0.5)) * scale - np.float32(0.5)
    src = np.maximum(src, np.float32(0.0))
    i0 = np.minimum(np.floor(src).astype(np.int32), in_size - 1)
    i1 = np.minimum(i0 + 1, in_size - 1)
    w1 = src - i0.astype(np.float32)
    return i0, i1, w1


def _klo_magic(g):
    """round-to-even(7g - 0.5) in strict fp32 — identical to the device chain."""
    t = (np.float32(7.0) * g - np.float32(0.5)).astype(np.float32)
    return ((t + MAGIC) - MAGIC).astype(np.float32)


def _host_prep(grid, guidance):
    grid = np.ascontiguousarray(grid, dtype=np.float32)
    guidance = np.ascontiguousarray(guidance, dtype=np.float32)

    y0, y1, wy = _src_coords(HH, HG)   # (1080,)
    x0, x1, wx = _src_coords(WH, WG)   # (1920,)

    # --- per-row y-lerped tables ------------------------------------------
    g0 = grid[:, :, :, y0, :]                      # (B, C, D, HH, WG)
    g1 = grid[:, :, :, y1, :]
    wyb = wy[None, None, None, :, None].astype(np.float32)
    T_all = ((np.float32(1.0) - wyb) * g0 + wyb * g1).astype(np.float32)
    T_all = np.transpose(T_all, (0, 3, 1, 2, 4)).reshape(ROWS_TOTAL, C, D, WG)

    TD = np.zeros_like(T_all)
    TD[:, :, : D - 1, :] = T_all[:, :, 1:, :] - T_all[:, :, : D - 1, :]
    jn = np.minimum(np.arange(WG) + 1, WG - 1)
    DjT = T_all[:, :, :, jn] - T_all
    DjTD = TD[:, :, :, jn] - TD

    # T4[row, e=(k,j), 64]: [c,4]-interleaved chunks
    T4 = np.zeros((ROWS_TOTAL, NE, ELEM), dtype=np.float32)
    q = np.stack([T_all, DjT, TD, DjTD], axis=-1)      # (rows, C, D, WG, 4)
    q = np.transpose(q, (0, 2, 3, 1, 4))               # (rows, D, WG, C, 4)
    T4[:, :, : C * 4] = q.reshape(ROWS_TOTAL, NE, C * 4)

    # --- pixel-stream quantities ------------------------------------------
    guid_rows = guidance[:, 0].reshape(ROWS_TOTAL, WH)

    # static per-stream-position values (same for every block)
    xs = np.tile(np.arange(WH, dtype=np.int64), BLK)       # x of stream pos
    gs = np.repeat(np.arange(BLK, dtype=np.int64), WH)     # row-in-block

    wx_pp = np.empty((128, NB), dtype=np.float32)          # wx per (p, slot)
    i_of = np.arange(NPIX)
    wx_pp[i_of % 128, i_of // 128] = wx[xs]

    t4_cores, idx_cores, gpp_cores = [], [], []
    for core in range(NCORES):
        r0 = core * RPC

        t4c = np.zeros((NBLK, BLK * NE, ELEM), dtype=np.float32)
        idxc = np.full((128, NBLK * (NPIX // 16)), -1, dtype=np.int16)
        gppc = np.zeros((128, NBLK * NB), dtype=np.float32)
        for t in range(NBLK):
            g_active = min(BLK, RPC - t * BLK)
            rows = np.arange(r0 + t * BLK, r0 + t * BLK + g_active)
            t4c[t, : g_active * NE] = T4[rows].reshape(g_active * NE, ELEM)

            n_act = g_active * WH
            gv = guid_rows[rows].reshape(-1)               # (n_act,) stream order
            klo = _klo_magic(gv)
            idx = (gs[:n_act] * NE + 16 * klo.astype(np.int64)
                   + x0[xs[:n_act]]).astype(np.int16)
            # wrapped (q = i%16, s = i//16), replicated across the 8 bands
            iw = np.full(NPIX, -1, np.int16)
            iw[:n_act] = idx
            wrapped = iw.reshape(NPIX // 16, 16).T         # (16, 960)
            for a in range(8):
                idxc[a * 16 : (a + 1) * 16, t * (NPIX // 16) : (t + 1) * (NPIX // 16)] = wrapped

            gpp = np.zeros(NPIX, np.float32)
            gpp[:n_act] = gv
            gppc[i_of % 128, t * NB + i_of // 128] = gpp
        t4_cores.append(t4c)
        idx_cores.append(idxc)
        gpp_cores.append(gppc)

    ident = np.eye(128, dtype=np.float32)
    return t4_cores, idx_cores, gpp_cores, wx_pp, ident


# ----------------------------------------------------------------------------
# Bass program (SPMD, one program for all 8 cores)
# ----------------------------------------------------------------------------

_NC_CACHE = None


def _build_nc():
    global _NC_CACHE
    if _NC_CACHE is not None:
        return _NC_CACHE

    NW = NBLK * (NPIX // 16)   # idx cols: 34*960
    NG = NBLK * NB             # guid cols: 34*120
    nc = bacc.Bacc("TRN2", target_bir_lowering=False, debug=True)

    t4_in = nc.dram_tensor("t4", [NBLK, BLK * NE, ELEM], F32, kind="ExternalInput")
    idx_in = nc.dram_tensor("idx", [128, NW], I16, kind="ExternalInput")
    gpp_in = nc.dram_tensor("gpp", [128, NG], F32, kind="ExternalInput")
    wx_in = nc.dram_tensor("wx", [128, NB], F32, kind="ExternalInput")
    id_in = nc.dram_tensor("ident", [128, 128], F32, kind="ExternalInput")
    out_d = nc.dram_tensor("out", [NBLK, NB, C, 128], F32, kind="ExternalOutput")

    ALU = mybir.AluOpType

    with tile.TileContext(nc) as tc:
        nc.gpsimd.load_library(library_config.mlp)

        with (
            tc.tile_pool(name="static", bufs=1) as statics,
            tc.tile_pool(name="guid", bufs=1) as guid_pool,
            tc.tile_pool(name="idxp", bufs=2) as idx_pool,
            tc.tile_pool(name="epool", bufs=2) as epool,
            tc.tile_pool(name="work", bufs=2) as work,
            tc.tile_pool(name="inter", bufs=1) as inter,
            tc.tile_pool(name="psum", bufs=2, space="PSUM") as psum,
        ):
            wx_t = statics.tile([128, NB], F32, tag="wx")
            id_t = statics.tile([128, 128], F32, tag="id")
            nc.sync.dma_start(out=wx_t[:], in_=wx_in[:])
            nc.sync.dma_start(out=id_t[:], in_=id_in[:])

            # ---- guidance -> f (all blocks at once) ------------------------
            gpp_t = guid_pool.tile([128, NG], F32, tag="gpp")
            f_t = guid_pool.tile([128, NG], F32, tag="f")
            nc.sync.dma_start(out=gpp_t[:], in_=gpp_in[:])
            nc.vector.tensor_scalar(
                out=f_t[:], in0=gpp_t[:], scalar1=7.0, scalar2=0.5,
                op0=ALU.mult, op1=ALU.subtract)
            nc.vector.tensor_scalar(
                out=f_t[:], in0=f_t[:], scalar1=float(MAGIC), scalar2=float(MAGIC),
                op0=ALU.add, op1=ALU.subtract)
            # f = 7g - klo   (in place: f_t currently holds klo)
            nc.vector.scalar_tensor_tensor(
                out=f_t[:], in0=gpp_t[:], scalar=7.0, in1=f_t[:],
                op0=ALU.mult, op1=ALU.subtract)

            # ---- main block loop -------------------------------------------
            for t in range(NBLK):
                g_active = min(BLK, RPC - t * BLK)
                npix = g_active * WH
                nb = npix // 128

                idx_t = idx_pool.tile([128, NPIX // 16], I16, tag="idx")
                nc.sync.dma_start(
                    out=idx_t[:, : npix // 16],
                    in_=idx_in[:, t * (NPIX // 16) : t * (NPIX // 16) + npix // 16])

                e_t = epool.tile([128, NB, ELEM], F32, tag="E")
                CH = 1024  # SDMA ring holds 64 descs/lane -> <=1024 idx/call
                for c0 in range(0, npix, CH):
                    cn = min(CH, npix - c0)
                    nc.gpsimd.dma_gather(
                        e_t[:, c0 // 128 : (c0 + cn) // 128, :], t4_in[t],
                        idx_t[:, c0 // 16 : (c0 + cn) // 16],
                        num_idxs=cn, num_idxs_reg=cn, elem_size=ELEM)

                tv = e_t[:, :nb, 0 : 4 * C : 4]
                djt = e_t[:, :nb, 1 : 4 * C : 4]
                tdv = e_t[:, :nb, 2 : 4 * C : 4]
                djtd = e_t[:, :nb, 3 : 4 * C : 4]
                wxv = wx_t[:, :nb].unsqueeze(2).broadcast_to((128, nb, C))
                fv = (f_t[:, t * NB : t * NB + nb]
                      .unsqueeze(2).broadcast_to((128, nb, C)))

                m1 = inter.tile([128, NB, C], F32, tag="m1")
                p_t = inter.tile([128, NB, C], F32, tag="P")
                q_t = inter.tile([128, NB, C], F32, tag="Q")
                # O laid out (p, c, b) so each channel is a contiguous 128-free
                # slab for the PE transpose.
                o_t = work.tile([128, C, NB], F32, tag="O")
                o_v = o_t[:].rearrange("p c b -> p b c")

                nc.vector.tensor_tensor(out=m1[:, :nb], in0=djt, in1=wxv, op=ALU.mult)
                nc.vector.tensor_tensor(out=p_t[:, :nb], in0=tv, in1=m1[:, :nb], op=ALU.add)
                nc.vector.tensor_tensor(out=m1[:, :nb], in0=djtd, in1=wxv, op=ALU.mult)
                nc.vector.tensor_tensor(out=q_t[:, :nb], in0=tdv, in1=m1[:, :nb], op=ALU.add)
                nc.vector.tensor_tensor(out=q_t[:, :nb], in0=q_t[:, :nb], in1=fv, op=ALU.mult)
                nc.vector.tensor_tensor(out=o_v[:, :nb], in0=p_t[:, :nb], in1=q_t[:, :nb], op=ALU.add)

                # ---- PE transpose per channel: (128, nb) -> (nb, 128) ------
                tr = psum.tile([NB, C * 128], F32, tag="tr")
                for c in range(C):
                    nc.tensor.transpose(
                        tr[:nb, c * 128 : (c + 1) * 128], o_t[:, c, :nb], id_t[:])
                s_t = work.tile([NB, C * 128], F32, tag="S")
                nc.scalar.copy(out=s_t[:nb], in_=tr[:nb])

                nc.sync.dma_start(
                    out=out_d[t, :nb],
                    in_=s_t[:nb].rearrange("b (c p) -> b c p", c=C))

    nc.finalize()
    _NC_CACHE = nc
    return nc


def kernel(grid, guidance):
    global LAST_EXEC_NS, LAST_PROFILE
    grid = np.asarray(grid, dtype=np.float32)
    guidance = np.asarray(guidance, dtype=np.float32)

    t4_cores, idx_cores, gpp_cores, wx_pp, ident = _host_prep(grid, guidance)

    nc = _build_nc()
    in_maps = []
    for core in range(NCORES):
        in_maps.append({
            "t4": t4_cores[core],
            "idx": idx_cores[core],
            "gpp": gpp_cores[core],
            "wx": wx_pp,
            "ident": ident,
        })

    trace = bool(int(os.environ.get("KTRACE", "0")))
    res = run_bass_kernel_spmd(nc, in_maps, core_ids=list(range(NCORES)),
                               trace=trace)
    LAST_EXEC_NS = res.exec_time_ns
    LAST_PROFILE = res.profile_json

    out = np.empty((B, C, HH, WH), dtype=np.float32)
    for core in range(NCORES):
        o = np.asarray(res.results[core]["out"])          # (NBLK, NB, C, 128)
        # pixel (p-lane, slot b, block t): row = t*8 + b//15, x = (b%15)*128 + p
        o = o.reshape(NBLK, BLK, XB, C, 128)              # (34, g, xb, c, lane)
        o = np.transpose(o, (3, 0, 1, 2, 4))              # (c, 34, g, xb, lane)
        o = o.reshape(C, NBLK * BLK, WH)[:, :RPC]         # (12, 270, 1920)
        r0 = core * RPC
        b = r0 // HH
        y0_ = r0 % HH
        out[b, :, y0_ : y0_ + RPC, :] = o
    return out
